# revision 6
# baseline (speedup 1.0000x reference)
"""Cantor global attention kernel for Trainium2 (8 NeuronCores, SPMD).

Strategy: data-parallel over batch B=64 -> 8 cores x 8 rows each.
Per core, every expert slab [8, 4096] is flattened to SBUF [128, 256]
(partition = b*16 + p//256, col = p%256) and all 16 experts sit side by
side in the free dimension.  The W=3 neighbor gather and the beta/
temperature gating are folded into per-(e,w) instruction operand offsets
and exp-activation scale immediates, baked at build time from the runtime
routes/betas/temperature values (tiny [16,3] control-plane tensors).

Math per expert e (projection mean + softmax over the 3 neighbors):
  Qs = q0+q1, Ks = k0+k1, Vs = v0+v1            (sums, not means)
  t_w  = Qs_e * Ks_{j_w}
  e_w  = exp(c_ew * t_w)      c_ew = 0.25*gate_ew/(sqrt(128)*|temp|)
  out  = (sum_w e_w*Vs_{j_w}) * 0.5 / (sum_w e_w)
The factor 0.25 folds both projection means of Q,K; the 0.5 folds V's.
"""

import numpy as np

import concourse.bass as bass
import concourse.mybir as mybir
from concourse import bacc, tile
from concourse.bass_utils import run_bass_kernel_spmd

E, NPROJ, B, P = 16, 2, 64, 4096
W = 3
EXPERT_DIM = 128
NCORES = 8
BS = B // NCORES          # 8 batch rows per core
COLS = 256                # free-dim columns per expert slab
PH = P // COLS            # 16 partition sub-blocks per batch row
PART = BS * PH            # 128 SBUF partitions
GROUP = 4                 # experts per DMA / tile group (1 MiB loads)
NG = E // GROUP           # 4 groups

F32 = mybir.dt.float32


def _build_nc(routes: np.ndarray, coef: np.ndarray):
    """Build + schedule the SPMD program with routes/coefs baked in."""
    nc = bacc.Bacc("TRN2", target_bir_lowering=False, debug=False,
                   num_devices=NCORES)

    q_d = nc.dram_tensor("q", [E, NPROJ, BS, P], F32, kind="ExternalInput")
    k_d = nc.dram_tensor("k", [E, NPROJ, BS, P], F32, kind="ExternalInput")
    v_d = nc.dram_tensor("v", [E, NPROJ, BS, P], F32, kind="ExternalInput")
    o_d = nc.dram_tensor("out", [BS, E * P], F32, kind="ExternalOutput")

    # DRAM views: [(b ph), e, n, c] so one dma_start covers a whole group.
    def load_view(t):
        return t.ap().rearrange("e n b (ph c) -> (b ph) e n c", c=COLS)

    qv, kv, vv = load_view(q_d), load_view(k_d), load_view(v_d)
    # Output view: [b, ph, e, c] (iteration order matches SBUF (part,e,c)).
    ov = o_d.ap().rearrange("b (e ph c) -> b ph e c", ph=PH, c=COLS)

    # Expert -> (group, col slice) in the group-sum tiles.
    def esl(j):
        return j // GROUP, slice((j % GROUP) * COLS, (j % GROUP + 1) * COLS)

    # Process experts in the order their gathered inputs become loadable.
    ready = [max(e // GROUP, max(routes[e]) // GROUP) for e in range(E)]
    order = sorted(range(E), key=lambda e: (ready[e], e))

    with tile.TileContext(nc) as tc:
        with (
            tc.tile_pool(name="rawq", bufs=2) as rawq_p,
            tc.tile_pool(name="rawk", bufs=2) as rawk_p,
            tc.tile_pool(name="rawv", bufs=2) as rawv_p,
            tc.tile_pool(name="qsum", bufs=NG) as qsum_p,
            tc.tile_pool(name="ksum", bufs=NG) as ksum_p,
            tc.tile_pool(name="vsum", bufs=NG) as vsum_p,
            tc.tile_pool(name="t3", bufs=3) as t3_p,
            tc.tile_pool(name="e3", bufs=3) as e3_p,
            tc.tile_pool(name="prod", bufs=3) as prod_p,
            tc.tile_pool(name="small", bufs=6) as small_p,
            tc.tile_pool(name="og", bufs=4) as og_p,
        ):
            qsum, ksum, vsum = [], [], []
            # ---- load + average, one group at a time ----
            for g in range(NG):
                es = slice(g * GROUP, (g + 1) * GROUP)
                for raw_p, dview, sums, s_p, eng in (
                    (rawq_p, qv, qsum, qsum_p, nc.sync),
                    (rawk_p, kv, ksum, ksum_p, nc.sync),
                    (rawv_p, vv, vsum, vsum_p, nc.scalar),
                ):
                    raw = raw_p.tile([PART, GROUP * NPROJ * COLS], F32)
                    rv = raw[:].rearrange("p (e n c) -> p e n c",
                                          e=GROUP, n=NPROJ)
                    eng.dma_start(rv, dview[:, es])
                    s = s_p.tile([PART, GROUP * COLS], F32)
                    sv = s[:].rearrange("p (e c) -> p e c", e=GROUP)
                    nc.vector.tensor_add(sv, rv[:, :, 0], rv[:, :, 1])
                    sums.append(s)

            # ---- per-expert attention ----
            og_tiles = {}
            done_in_group = [0] * NG
            for e in order:
                ge, se = esl(e)
                t3 = t3_p.tile([PART, W * COLS], F32)
                e3 = e3_p.tile([PART, W * COLS], F32)
                prod = prod_p.tile([PART, W * COLS], F32)
                for w in range(W):
                    j = int(routes[e, w])
                    gj, sj = esl(j)
                    wsl = slice(w * COLS, (w + 1) * COLS)
                    nc.vector.tensor_mul(t3[:, wsl], qsum[ge][:, se],
                                         ksum[gj][:, sj])
                    nc.scalar.activation(e3[:, wsl], t3[:, wsl],
                                         mybir.ActivationFunctionType.Exp,
                                         bias=0.0, scale=float(coef[e, w]))
                    nc.gpsimd.tensor_mul(prod[:, wsl], e3[:, wsl],
                                         vsum[gj][:, sj])
                den = small_p.tile([PART, COLS], F32, tag="den")
                nc.vector.tensor_add(den[:], e3[:, 0:COLS], e3[:, COLS:2 * COLS])
                nc.vector.tensor_add(den[:], den[:], e3[:, 2 * COLS:3 * COLS])
                rcp = small_p.tile([PART, COLS], F32, tag="rcp")
                nc.vector.reciprocal(rcp[:], den[:])
                num = small_p.tile([PART, COLS], F32, tag="num")
                nc.vector.tensor_add(num[:], prod[:, 0:COLS],
                                     prod[:, COLS:2 * COLS])
                nc.vector.tensor_add(num[:], num[:], prod[:, 2 * COLS:3 * COLS])
                og = og_p.tile([PART, COLS], F32, name="og", tag="og")
                # out = (num * 0.5) * (1/den)
                nc.vector.scalar_tensor_tensor(
                    og[:], num[:], 0.5, rcp[:],
                    mybir.AluOpType.mult, mybir.AluOpType.mult)
                nc.sync.dma_start(ov[:, :, e], og[:])

    nc.compile()
    return nc


_cache: dict = {}


def _get_nc(routes: np.ndarray, coef: np.ndarray):
    key = (routes.tobytes(), coef.tobytes())
    if key not in _cache:
        _cache[key] = _build_nc(routes, coef)
    return _cache[key]


def kernel(Q_proj, K_proj, V_proj, betas, temperature, routes, num_patches):
    Q = np.asarray(Q_proj, dtype=np.float32)
    K = np.asarray(K_proj, dtype=np.float32)
    V = np.asarray(V_proj, dtype=np.float32)
    betas = np.asarray(betas, dtype=np.float32)
    temp = np.asarray(temperature, dtype=np.float32)
    routes = np.asarray(routes, dtype=np.int32)

    # Host control-plane: beta gating + scale folded into one coefficient
    # per (expert, neighbor).  0.25 = the two projection means of Q and K.
    scale = np.float32(np.sqrt(np.float32(EXPERT_DIM))) * np.abs(temp[0])
    gate = np.where(routes != np.arange(E, dtype=np.int32)[:, None],
                    np.float32(1.0) / (np.float32(1.0) + np.exp(-betas)),
                    np.float32(1.0)).astype(np.float32)
    coef = (np.float32(0.25) * gate / scale).astype(np.float32)

    nc = _get_nc(routes, coef)
    in_maps = [
        {
            "q": np.ascontiguousarray(Q[:, :, c * BS:(c + 1) * BS, :]),
            "k": np.ascontiguousarray(K[:, :, c * BS:(c + 1) * BS, :]),
            "v": np.ascontiguousarray(V[:, :, c * BS:(c + 1) * BS, :]),
        }
        for c in range(NCORES)
    ]
    res = run_bass_kernel_spmd(nc, in_maps, list(range(NCORES)))
    return np.concatenate([res.results[c]["out"] for c in range(NCORES)],
                          axis=0)


# revision 10
# speedup vs baseline: 1.2933x; 1.2933x over previous
"""Cantor global attention kernel for Trainium2 (8 NeuronCores, SPMD).

Strategy: data-parallel over batch B=64 -> 8 cores x 8 rows each.
Per core, every expert slab [8, 4096] is flattened to SBUF [128, 256]
(partition = b*16 + p//256, col = p%256); experts sit side by side in
the free dimension, grouped 4 per tile.  The W=3 neighbor gather and
the beta/temperature gating are folded into per-(e,w) instruction
operand offsets and exp-activation scale immediates, baked at build
time from the runtime routes/betas/temperature values (tiny [16,3]
control-plane tensors).

Work placement (per core, all f32):
  - projection averaging: DMA-accumulate (CCE add in the SDMA engines),
    zero compute-engine cost; the 2x is folded into downstream scales
  - t_w = Qs*Ks:      DVE tensor_mul, run-batched over consecutive-route
                      spans so one instruction covers several experts
  - e_w = exp(c*t):   ScalarE activation, scale=c_ew immediate (in-place)
  - prod_w = e_w*Vs:  split DVE / GpSimd (lane-tunable)
  - den = sum_w e_w:  DVE adds;  r = 0.5/den = exp(-ln(den)+ln(.5)): ACT
  - num = sum prod:   DVE adds;  out = num*r: DVE mul
"""

import math

import numpy as np

import concourse.bass as bass
import concourse.mybir as mybir
from concourse import bacc, tile
from concourse.bass_utils import run_bass_kernel_spmd

E, NPROJ, B, P = 16, 2, 64, 4096
W = 3
EXPERT_DIM = 128
NCORES = 8
BS = B // NCORES          # 8 batch rows per core
COLS = 256                # free-dim columns per expert slab
PH = P // COLS            # 16 partition sub-blocks per batch row
PART = BS * PH            # 128 SBUF partitions
GROUP = 4                 # experts per tile group
NG = E // GROUP           # 4 groups
GC = GROUP * COLS         # 1024 cols per group tile

F32 = mybir.dt.float32
EXPF = mybir.ActivationFunctionType.Exp
LNF = mybir.ActivationFunctionType.Ln

# prod lanes routed to GpSimd (rest on DVE); num adds engine
GP_PROD_W = (1, 2)
NUM_ON_GP = False


def _runs(pairs):
    """Split [(le, j), ...] into maximal runs with consecutive le and j
    within one j-group."""
    runs = []
    for le, j in pairs:
        if (runs and runs[-1][0] + runs[-1][2] == le
                and runs[-1][1] + runs[-1][2] == j
                and (runs[-1][1] // GROUP == j // GROUP)):
            runs[-1][2] += 1
        else:
            runs.append([le, j, 1])
    return runs


def _build_nc(routes: np.ndarray, coef: np.ndarray):
    nc = bacc.Bacc("TRN2", target_bir_lowering=False, debug=False,
                   num_devices=NCORES)

    q_d = nc.dram_tensor("q", [E, NPROJ, BS, P], F32, kind="ExternalInput")
    k_d = nc.dram_tensor("k", [E, NPROJ, BS, P], F32, kind="ExternalInput")
    v_d = nc.dram_tensor("v", [E, NPROJ, BS, P], F32, kind="ExternalInput")
    o_d = nc.dram_tensor("out", [BS, E * P], F32, kind="ExternalOutput")

    # DRAM views: [(b ph), e, n, c]
    def lview(t):
        return t.ap().rearrange("e n b (ph c) -> (b ph) e n c", c=COLS)

    qv, kv, vv = lview(q_d), lview(k_d), lview(v_d)
    ov = o_d.ap().rearrange("b (e ph c) -> b ph e c", ph=PH, c=COLS)

    # group g of experts is ready once groups up to ready_g[g] are loaded
    ready_g = [max(g, int(routes[g * GROUP:(g + 1) * GROUP].max()) // GROUP)
               for g in range(NG)]

    with tile.TileContext(nc) as tc:
        with (
            tc.tile_pool(name="qs", bufs=NG) as qs_p,
            tc.tile_pool(name="ks", bufs=NG) as ks_p,
            tc.tile_pool(name="vs", bufs=NG) as vs_p,
            tc.tile_pool(name="te3", bufs=2) as te3_p,
            tc.tile_pool(name="p3", bufs=2) as p3_p,
            tc.tile_pool(name="sm", bufs=2) as sm_p,
        ):
            qs, ks, vs = [], [], []
            lnhalf = sm_p.tile([PART, 1], F32, name="lnhalf", tag="lnhalf",
                               bufs=1)
            nc.gpsimd.memset(lnhalf[:], math.log(0.5))

            def emit_group(g):
                """All compute + stores for expert group g."""
                e0 = g * GROUP
                te3 = te3_p.tile([PART, W * GC], F32, name="te3", tag="te3")
                p3 = p3_p.tile([PART, W * GC], F32, name="p3", tag="p3")
                for w in range(W):
                    pairs = [(le, int(routes[e0 + le, w]))
                             for le in range(GROUP)]
                    # t_w = Qs * Ks[route]   (run-batched)
                    for le, j, L in _runs(pairs):
                        gj, lj = j // GROUP, j % GROUP
                        dst = te3[:, w * GC + le * COLS:
                                  w * GC + (le + L) * COLS]
                        nc.vector.tensor_mul(
                            dst,
                            qs[g][:, le * COLS:(le + L) * COLS],
                            ks[gj][:, lj * COLS:(lj + L) * COLS])
                    # e_w = exp(c_ew * t_w)   in-place, per expert
                    for le in range(GROUP):
                        sl = slice(w * GC + le * COLS,
                                   w * GC + (le + 1) * COLS)
                        nc.scalar.activation(te3[:, sl], te3[:, sl], EXPF,
                                             bias=0.0,
                                             scale=float(coef[e0 + le, w]))
                    # prod_w = e_w * Vs[route]   (run-batched, split engines)
                    eng = nc.gpsimd if w in GP_PROD_W else nc.vector
                    for le, j, L in _runs(pairs):
                        gj, lj = j // GROUP, j % GROUP
                        sl = slice(w * GC + le * COLS,
                                   w * GC + (le + L) * COLS)
                        eng.tensor_mul(p3[:, sl], te3[:, sl],
                                       vs[gj][:, lj * COLS:(lj + L) * COLS])
                # den = sum_w e_w ; r = 0.5/den = exp(-ln(den) + ln(0.5))
                den = sm_p.tile([PART, GC], F32, name="den", tag="den")
                nc.vector.tensor_add(den[:], te3[:, 0:GC], te3[:, GC:2 * GC])
                nc.vector.tensor_add(den[:], den[:], te3[:, 2 * GC:3 * GC])
                nc.scalar.activation(den[:], den[:], LNF)
                rcp = sm_p.tile([PART, GC], F32, name="rcp", tag="rcp")
                nc.scalar.activation(rcp[:], den[:], EXPF,
                                     bias=lnhalf[:], scale=-1.0)
                # num = sum_w prod_w ; out = num * r
                num = sm_p.tile([PART, GC], F32, name="num", tag="num")
                neng = nc.gpsimd if NUM_ON_GP else nc.vector
                neng.tensor_add(num[:], p3[:, 0:GC], p3[:, GC:2 * GC])
                neng.tensor_add(num[:], num[:], p3[:, 2 * GC:3 * GC])
                og = sm_p.tile([PART, GC], F32, name="og", tag="og")
                nc.vector.tensor_mul(og[:], num[:], rcp[:])
                for le in range(GROUP):
                    nc.sync.dma_start(ov[:, :, e0 + le],
                                      og[:, le * COLS:(le + 1) * COLS])

            emitted = [False] * NG
            for g in range(NG):
                es = slice(g * GROUP, (g + 1) * GROUP)
                for dview, sums, s_p, eng in (
                    (qv, qs, qs_p, nc.sync),
                    (kv, ks, ks_p, nc.sync),
                    (vv, vs, vs_p, nc.scalar),
                ):
                    s = s_p.tile([PART, GC], F32, name="s", tag="s")
                    eng.dma_start(s[:], dview[:, es, 0])
                    nc.gpsimd.dma_start(s[:], dview[:, es, 1],
                                        accum_op=mybir.AluOpType.add)
                    sums.append(s)
                for g2 in range(NG):
                    if not emitted[g2] and ready_g[g2] <= g:
                        emit_group(g2)
                        emitted[g2] = True

    nc.compile()
    return nc


_cache: dict = {}


def _get_nc(routes: np.ndarray, coef: np.ndarray):
    key = (routes.tobytes(), coef.tobytes())
    if key not in _cache:
        _cache[key] = _build_nc(routes, coef)
    return _cache[key]


def kernel(Q_proj, K_proj, V_proj, betas, temperature, routes, num_patches):
    Q = np.asarray(Q_proj, dtype=np.float32)
    K = np.asarray(K_proj, dtype=np.float32)
    V = np.asarray(V_proj, dtype=np.float32)
    betas = np.asarray(betas, dtype=np.float32)
    temp = np.asarray(temperature, dtype=np.float32)
    routes = np.asarray(routes, dtype=np.int32)

    # Host control-plane: beta gating + scale folded into one coefficient
    # per (expert, neighbor).  0.25 = the two projection means of Q and K
    # (sums are averaged); V's 0.5 is folded into the reciprocal's bias.
    scale = np.float32(np.sqrt(np.float32(EXPERT_DIM))) * np.abs(temp[0])
    gate = np.where(routes != np.arange(E, dtype=np.int32)[:, None],
                    np.float32(1.0) / (np.float32(1.0) + np.exp(-betas)),
                    np.float32(1.0)).astype(np.float32)
    coef = (np.float32(0.25) * gate / scale).astype(np.float32)

    nc = _get_nc(routes, coef)
    in_maps = [
        {
            "q": np.ascontiguousarray(Q[:, :, c * BS:(c + 1) * BS, :]),
            "k": np.ascontiguousarray(K[:, :, c * BS:(c + 1) * BS, :]),
            "v": np.ascontiguousarray(V[:, :, c * BS:(c + 1) * BS, :]),
        }
        for c in range(NCORES)
    ]
    res = run_bass_kernel_spmd(nc, in_maps, list(range(NCORES)))
    return np.concatenate([res.results[c]["out"] for c in range(NCORES)],
                          axis=0)
